# revision 5
# baseline (speedup 1.0000x reference)
"""Self-contained TRN2 Bass kernel for the GCN message-passing problem.

8-core SPMD, v4:
- Nodes sharded by dst across cores (NS = N/C contiguous nodes per core).
- Table t[v] = h[v] (bf16, node-major [N, 64]) replicated in every core's
  DRAM. Layer0 table is computed locally (replicated embed, no collective);
  tables for layers 1,2 are distributed via CHUNKED AllGathers interleaved
  with block processing (producer-side overlap).
- Edges (incl. self-loops) grouped by dst block (128 dsts), padded to tiles
  of 128 edges. Per tile: one indirect-DMA gather (128 random bf16 rows),
  a one-hot selection matrix with the full GCN norm (dinv_src*dinv_dst)
  folded in (DVE tensor_scalar is_equal+mult, bf16), and a PE matmul
  psum[64f,128d] += gather[128e,64f].T @ onehot[128e,128d].
- Per-block conv transform: lhsT = hagg1[65, 128] (row 64 = ones) x
  cw1[65, 64] (row 64 = bias) -> psum[128n, 64f] node-major; ACT relu ->
  bf16 tile -> DMA to bounce (AllGather source) or h3n (last layer).
- Pooling: one-hot over G graphs into psum [64, G], AllReduce, tiny MLP
  on every core; core 0's output used.
"""
from dataclasses import dataclass

import numpy as np
import jax
from jax.sharding import Mesh, PartitionSpec
from jax.experimental.shard_map import shard_map

from concourse import bass2jax
from concourse.bass2jax import _bass_exec_p, install_neuronx_cc_hook

import concourse.bass as bass
import concourse.bacc as bacc
import concourse.mybir as mybir
import concourse.tile as tile

F32 = mybir.dt.float32
BF16 = mybir.dt.bfloat16
I32 = mybir.dt.int32


@dataclass
class Meta:
    N: int
    F: int
    H: int
    G: int
    L: int
    C: int
    NS: int
    NB: int
    NBF: int
    tbb: tuple
    toff: tuple
    T_tot: int
    NCHK: int
    BPC: int
    nq: int = 4


def preprocess(x, edge_index, batch, W_emb, b_emb, conv_W, conv_b,
               W1, b1, W2, b2, W3, b3, n_cores=8, G=None, NCHK=5, nq=4):
    """Host-side index preprocessing. Returns (meta, in_maps)."""
    x = np.asarray(x, np.float32)
    ei = np.asarray(edge_index, np.int64)
    batch = np.asarray(batch, np.int64)
    N, F = x.shape
    H = int(np.asarray(W_emb).shape[1])
    L = int(np.asarray(conv_W).shape[0])
    C = n_cores
    assert N % C == 0
    NS = N // C
    NB = (NS + 127) // 128
    NBF = (N + 127) // 128
    if G is None:
        G = int(batch.max()) + 1 if batch.size else 1

    loop = np.arange(N, dtype=np.int64)
    deg = (np.bincount(np.concatenate([ei[1], loop]), minlength=N)
           .astype(np.float64))
    dinv = (1.0 / np.sqrt(np.maximum(deg, 1.0))).astype(np.float32)
    # self-loops are ordinary edges
    src = np.concatenate([ei[0], loop])
    dst = np.concatenate([ei[1], loop])

    # sort edges by (core, block, src)
    core = dst // NS
    block = (dst % NS) // 128
    order = np.lexsort((src, block, core))
    src_s, dst_s = src[order], dst[order]
    core_s, block_s = core[order], block[order]

    cnt = np.zeros((C, NB), np.int64)
    np.add.at(cnt, (core_s, block_s), 1)
    tbb = np.maximum(1, (cnt.max(axis=0) + 127) // 128).astype(np.int64)
    toff = np.zeros(NB + 1, np.int64)
    toff[1:] = np.cumsum(tbb)
    T_tot = int(toff[-1])

    # chunked-AllGather table layout for layers 1..L-1
    BPC = (NB + NCHK - 1) // NCHK
    chunk_rows = []          # rows per core in chunk k
    base = []                # start row of chunk k in the table tensor
    acc = 0
    for k in range(NCHK):
        r0 = min(NS, k * BPC * 128)
        r1 = min(NS, (k + 1) * BPC * 128)
        chunk_rows.append(r1 - r0)
        base.append(acc)
        acc += C * (r1 - r0)
    assert acc == N

    def remap(v):
        c = v // NS
        r = v % NS
        k = np.minimum(r // (BPC * 128), NCHK - 1)
        kb = np.asarray(base)[k]
        rk = np.asarray(chunk_rows)[k]
        return (kb + c * rk + (r - k * (BPC * 128))).astype(np.int32)

    idx0_all = np.zeros((C, 128, T_tot), np.int32)
    idx12_all = np.zeros((C, 128, T_tot), np.int32)
    dstloc = np.full((C, 128, T_tot), -1.0, np.float32)
    norm_e = np.zeros((C, 128, T_tot), np.float32)

    starts = np.zeros(C * NB, np.int64)
    starts[1:] = np.cumsum(cnt.ravel())[:-1]
    starts = starts.reshape(C, NB)
    for c in range(C):
        for b in range(NB):
            n = int(cnt[c, b])
            if n == 0:
                continue
            s0 = int(starts[c, b])
            e_src = src_s[s0:s0 + n]
            e_dst = dst_s[s0:s0 + n]
            j = np.arange(n)
            t = int(toff[b]) + j // 128
            p = j % 128
            idx0_all[c, p, t] = e_src
            idx12_all[c, p, t] = remap(e_src)
            dstloc[c, p, t] = (e_dst - c * NS - b * 128).astype(np.float32)
            norm_e[c, p, t] = dinv[e_src] * dinv[e_dst]
            # pad gather idx with last valid src; dstloc stays -1 -> zero col
            if (n % 128) != 0:
                lastt = int(toff[b]) + (n - 1) // 128
                idx0_all[c, (n % 128):, lastt] = e_src[-1]
                idx12_all[c, (n % 128):, lastt] = remap(e_src[-1:])[0]
            for tt in range(int(toff[b]) + (n + 127) // 128, int(toff[b + 1])):
                idx0_all[c, :, tt] = e_src[-1]
                idx12_all[c, :, tt] = remap(e_src[-1:])[0]

    # replicated-embed inputs: x^T with a trailing ones row; W_emb with bias
    xT1 = np.ones((F + 1, N), np.float32)
    xT1[:F] = x.T
    xT1 = xT1.astype(jax.numpy.bfloat16)
    wemb1 = np.concatenate(
        [np.asarray(W_emb, np.float32),
         np.asarray(b_emb, np.float32).reshape(1, H)], axis=0)
    wemb1 = wemb1.astype(jax.numpy.bfloat16)

    conv_W = np.asarray(conv_W, np.float32)
    conv_b = np.asarray(conv_b, np.float32)
    cw1 = np.zeros((L, H + 1, H), np.float32)
    for i in range(L):
        cw1[i, :H] = conv_W[i]
        cw1[i, H] = conv_b[i]
    cw1 = cw1.astype(jax.numpy.bfloat16)

    iota128 = np.tile(np.arange(128, dtype=np.float32),
                      (128, 1)).astype(jax.numpy.bfloat16)
    iotag = np.tile(np.arange(G, dtype=np.float32),
                    (128, 1)).astype(jax.numpy.bfloat16)

    cntg = np.bincount(batch, minlength=G).astype(np.float32)
    invc = np.tile((1.0 / np.maximum(cntg, 1.0))[None, :],
                   (64, 1)).astype(np.float32)

    meta = Meta(N=N, F=F, H=H, G=G, L=L, C=C, NS=NS, NB=NB, NBF=NBF,
                tbb=tuple(int(v) for v in tbb),
                toff=tuple(int(v) for v in toff), T_tot=T_tot,
                NCHK=NCHK, BPC=BPC, nq=nq)

    in_maps = []
    for c in range(C):
        basep = c * NS
        poolid = np.full((128, NB), -1.0, np.float32)
        for b in range(NB):
            w = min(128, NS - b * 128)
            poolid[:w, b] = batch[basep + b * 128: basep + b * 128 + w]
        m = {
            "xt1": xT1,
            "idx0": np.ascontiguousarray(idx0_all[c]),
            "idx12": np.ascontiguousarray(idx12_all[c]),
            "dstloc": np.ascontiguousarray(dstloc[c]),
            "norme": np.ascontiguousarray(norm_e[c]),
            "poolid": poolid,
            "iota128": iota128,
            "iotag": iotag,
            "wemb1": wemb1,
            "invc": invc,
            "w1": np.asarray(W1, np.float32),
            "b1": np.asarray(b1, np.float32).reshape(-1, 1),
            "w2": np.asarray(W2, np.float32),
            "b2": np.asarray(b2, np.float32).reshape(-1, 1),
            "w3": np.asarray(W3, np.float32),
            "b3": np.asarray(b3, np.float32).reshape(1, 1),
        }
        for i in range(L):
            m[f"cw1_{i}"] = np.ascontiguousarray(cw1[i])
        in_maps.append(m)
    return meta, in_maps


def build_nc(meta: Meta, repeats=1):
    N, F, H, G, L, C = meta.N, meta.F, meta.H, meta.G, meta.L, meta.C
    NS, NB, NBF = meta.NS, meta.NB, meta.NBF
    tbb, toff, T_tot = meta.tbb, meta.toff, meta.T_tot
    NCHK, BPC, nq = meta.NCHK, meta.BPC, meta.nq

    nc = bacc.Bacc("TRN2", target_bir_lowering=False, debug=False,
                   num_devices=C, num_swdge_queues=max(nq, 1))
    qnames = ["qPoolDynamic"] + [f"qPoolDynamic{i}" for i in range(1, nq)]

    def EIN(name, shape, dt):
        return nc.dram_tensor(name, list(shape), dt, kind="ExternalInput")

    xt1 = EIN("xt1", [F + 1, N], BF16)
    idx0 = EIN("idx0", [128, T_tot], I32)
    idx12 = EIN("idx12", [128, T_tot], I32)
    dstloc = EIN("dstloc", [128, T_tot], F32)
    norme = EIN("norme", [128, T_tot], F32)
    poolid = EIN("poolid", [128, NB], F32)
    iota128 = EIN("iota128", [128, 128], BF16)
    iotag = EIN("iotag", [128, G], BF16)
    wemb1 = EIN("wemb1", [F + 1, H], BF16)
    invc = EIN("invc", [64, G], F32)
    w1 = EIN("w1", [H, H], F32)
    b1 = EIN("b1", [H, 1], F32)
    w2 = EIN("w2", [H, H // 2], F32)
    b2 = EIN("b2", [H // 2, 1], F32)
    w3 = EIN("w3", [H // 2, 1], F32)
    b3 = EIN("b3", [1, 1], F32)
    cw1 = [EIN(f"cw1_{i}", [H + 1, H], BF16) for i in range(L)]

    out_d = nc.dram_tensor("out", [1, G], F32, kind="ExternalOutput")

    table0 = nc.dram_tensor("table0", [N, H], BF16)
    table1 = nc.dram_tensor("table1", [N, H], BF16, addr_space="Shared")
    table2 = nc.dram_tensor("table2", [N, H], BF16, addr_space="Shared")
    bounce = nc.dram_tensor("bounce", [NS, H], BF16)
    pool_in = nc.dram_tensor("pool_in", [H, G], F32)
    pool_out = nc.dram_tensor("pool_out", [H, G], F32, addr_space="Shared")

    groups = [list(range(C))]

    # chunk row ranges within the shard
    chunk_lim = [(min(NS, k * BPC * 128), min(NS, (k + 1) * BPC * 128))
                 for k in range(NCHK)]
    chunk_base = []
    acc = 0
    for k in range(NCHK):
        chunk_base.append(acc)
        acc += C * (chunk_lim[k][1] - chunk_lim[k][0])

    gq = [0]

    def gather(g_ap, t_in, idx_sb, tt):
        inst = nc.gpsimd.indirect_dma_start(
            out=g_ap, out_offset=None, in_=t_in[:],
            in_offset=bass.IndirectOffsetOnAxis(
                ap=idx_sb[:, tt:tt + 1], axis=0))
        if nq > 1:
            inst.ins.queue = qnames[gq[0] % nq]
            gq[0] += 1

    with tile.TileContext(nc) as tc:
        import contextlib
        ctx = contextlib.ExitStack()
        with ctx:
            P = ctx.enter_context
            persist = P(tc.tile_pool(name="persist", bufs=1))
            xpool = P(tc.tile_pool(name="xpool", bufs=3))
            gpool = P(tc.tile_pool(name="gpool", bufs=48))
            ohpool = P(tc.tile_pool(name="ohpool", bufs=24))
            stpool = P(tc.tile_pool(name="stpool", bufs=4))
            pohpool = P(tc.tile_pool(name="pohpool", bufs=3))
            bp_ps = P(tc.tile_pool(name="bp_ps", bufs=3, space="PSUM"))
            st_ps = P(tc.tile_pool(name="st_ps", bufs=3, space="PSUM"))

            def load(name, ap, shape, dt):
                t = persist.tile(list(shape), dt, tag=name)
                nc.sync.dma_start(out=t[:], in_=ap[:])
                return t

            idx0_sb = load("idx0_sb", idx0, [128, T_tot], I32)
            idx12_sb = load("idx12_sb", idx12, [128, T_tot], I32)
            dstloc_sb = load("dstloc_sb", dstloc, [128, T_tot], F32)
            norme_sb = load("norme_sb", norme, [128, T_tot], F32)
            poolid_sb = load("poolid_sb", poolid, [128, NB], F32)
            iota_sb = load("iota_sb", iota128, [128, 128], BF16)
            iotag_sb = load("iotag_sb", iotag, [128, G], BF16)
            wemb1_sb = load("wemb1_sb", wemb1, [F + 1, H], BF16)
            invc_sb = load("invc_sb", invc, [64, G], F32)
            w1_sb = load("w1_sb", w1, [H, H], F32)
            b1_sb = load("b1_sb", b1, [H, 1], F32)
            w2_sb = load("w2_sb", w2, [H, H // 2], F32)
            b2_sb = load("b2_sb", b2, [H // 2, 1], F32)
            w3_sb = load("w3_sb", w3, [H // 2, 1], F32)
            b3_sb = load("b3_sb", b3, [1, 1], F32)
            cw1_sb = [load(f"cw1_{i}_sb", cw1[i], [H + 1, H], BF16)
                      for i in range(L)]

            hagg1 = persist.tile([H + 1, NS], BF16, tag="hagg1")
            h3n = persist.tile([128, NB * H], BF16, tag="h3n")
            nc.vector.memset(hagg1[H:H + 1, :], 1.0)

            for _rep in range(repeats):
                # ================= embed (replicated, node-major) ============
                XCH = 512
                for c0 in range(0, N, XCH):
                    cwd = min(XCH, N - c0)
                    xt = xpool.tile([F + 1, XCH], BF16, tag="xt")
                    nc.sync.dma_start(out=xt[:, :cwd], in_=xt1[:, c0:c0 + cwd])
                    for o in range(0, cwd, 128):
                        w = min(128, cwd - o)
                        ps = st_ps.tile([128, H], F32, tag="stps")
                        nc.tensor.matmul(out=ps[:w, :], lhsT=xt[:, o:o + w],
                                         rhs=wemb1_sb[:], start=True, stop=True)
                        st = stpool.tile([128, H], BF16, tag="st")
                        nc.scalar.activation(
                            out=st[:w, :], in_=ps[:w, :],
                            func=mybir.ActivationFunctionType.Relu)
                        nc.sync.dma_start(
                            out=table0[c0 + o:c0 + o + w, :], in_=st[:w, :])

                # ================= conv layers =================
                tables = [table0, table1, table2]
                for li in range(L):
                    t_in = tables[li]
                    idx_sb = idx0_sb if li == 0 else idx12_sb
                    for b in range(NB):
                        w = min(128, NS - b * 128)
                        ps = bp_ps.tile([64, 128], F32, tag="bps")
                        for t in range(tbb[b]):
                            tt = toff[b] + t
                            g = gpool.tile([128, H], BF16, tag="g")
                            gather(g[:], t_in, idx_sb, tt)
                            oh = ohpool.tile([128, 128], BF16, tag="oh")
                            nc.vector.tensor_scalar(
                                out=oh[:], in0=iota_sb[:],
                                scalar1=dstloc_sb[:, tt:tt + 1],
                                scalar2=norme_sb[:, tt:tt + 1],
                                op0=mybir.AluOpType.is_equal,
                                op1=mybir.AluOpType.mult)
                            nc.tensor.matmul(
                                out=ps[:], lhsT=g[:], rhs=oh[:],
                                start=(t == 0), stop=(t == tbb[b] - 1))
                        nc.vector.tensor_copy(
                            out=hagg1[:H, b * 128:b * 128 + w],
                            in_=ps[:, :w])
                        # conv transform: node-major h' = relu(agg @ W + b)
                        ps2 = st_ps.tile([128, H], F32, tag="stps")
                        nc.tensor.matmul(
                            out=ps2[:w, :],
                            lhsT=hagg1[:, b * 128:b * 128 + w],
                            rhs=cw1_sb[li][:], start=True, stop=True)
                        if li < L - 1:
                            st = stpool.tile([128, H], BF16, tag="st")
                            nc.scalar.activation(
                                out=st[:w, :], in_=ps2[:w, :],
                                func=mybir.ActivationFunctionType.Relu)
                            nc.sync.dma_start(
                                out=bounce[b * 128:b * 128 + w, :],
                                in_=st[:w, :])
                        else:
                            nc.scalar.activation(
                                out=h3n[:w, b * H:(b + 1) * H],
                                in_=ps2[:w, :],
                                func=mybir.ActivationFunctionType.Relu)
                        # fire the chunk AllGather as soon as its blocks done
                        if li < L - 1:
                            for k in range(NCHK):
                                if b == min(NB, (k + 1) * BPC) - 1:
                                    r0, r1 = chunk_lim[k]
                                    if r1 > r0:
                                        t_out = tables[li + 1]
                                        kb = chunk_base[k]
                                        ke = kb + C * (r1 - r0)
                                        nc.gpsimd.collective_compute(
                                            "AllGather",
                                            mybir.AluOpType.bypass,
                                            replica_groups=groups,
                                            ins=[bounce[r0:r1, :]],
                                            outs=[t_out[kb:ke, :]])

                # ================= pooling =================
                with tc.tile_pool(name="pool_ps", bufs=1,
                                  space="PSUM") as pool_ps:
                    pps = pool_ps.tile([64, G], F32, tag="pps")
                    for b in range(NB):
                        w = min(128, NS - b * 128)
                        ohp = pohpool.tile([128, G], BF16, tag="ohp")
                        nc.vector.tensor_scalar(
                            out=ohp[:w, :], in0=iotag_sb[:w, :],
                            scalar1=poolid_sb[:w, b:b + 1], scalar2=None,
                            op0=mybir.AluOpType.is_equal)
                        nc.tensor.matmul(out=pps[:],
                                         lhsT=h3n[:w, b * H:(b + 1) * H],
                                         rhs=ohp[:w, :], start=(b == 0),
                                         stop=(b == NB - 1))
                    psum_sb = persist.tile([64, G], F32, tag="psum_sb")
                    nc.vector.tensor_copy(out=psum_sb[:], in_=pps[:])
                nc.sync.dma_start(out=pool_in[:], in_=psum_sb[:])
                nc.gpsimd.collective_compute(
                    "AllReduce", mybir.AluOpType.add, replica_groups=groups,
                    ins=[pool_in[:]], outs=[pool_out[:]])
                pooled = persist.tile([64, G], F32, tag="pooled")
                nc.sync.dma_start(out=pooled[:], in_=pool_out[:])
                nc.vector.tensor_tensor(out=pooled[:], in0=pooled[:],
                                        in1=invc_sb[:],
                                        op=mybir.AluOpType.mult)
                # ================= MLP =================
                with tc.tile_pool(name="mlp_ps", bufs=2,
                                  space="PSUM") as mlp_ps:
                    ps1 = mlp_ps.tile([64, G], F32, tag="mlpps")
                    nc.tensor.matmul(out=ps1[:, :G], lhsT=w1_sb[:],
                                     rhs=pooled[:], start=True, stop=True)
                    r1 = persist.tile([64, G], F32, tag="r1")
                    nc.scalar.activation(
                        out=r1[:], in_=ps1[:64, :G],
                        func=mybir.ActivationFunctionType.Relu,
                        bias=b1_sb[:, 0:1])
                    ps2m = mlp_ps.tile([64, G], F32, tag="mlpps")
                    nc.tensor.matmul(out=ps2m[:32, :G], lhsT=w2_sb[:],
                                     rhs=r1[:], start=True, stop=True)
                    r2 = persist.tile([32, G], F32, tag="r2")
                    nc.scalar.activation(
                        out=r2[:], in_=ps2m[:32, :G],
                        func=mybir.ActivationFunctionType.Relu,
                        bias=b2_sb[:, 0:1])
                    ps3 = mlp_ps.tile([64, G], F32, tag="mlpps")
                    nc.tensor.matmul(out=ps3[:1, :G], lhsT=w3_sb[:],
                                     rhs=r2[:], start=True, stop=True)
                    outs = persist.tile([1, G], F32, tag="outs")
                    nc.vector.tensor_scalar(out=outs[:], in0=ps3[:1, :G],
                                            scalar1=b3_sb[0:1, 0:1],
                                            scalar2=None,
                                            op0=mybir.AluOpType.add)
                nc.sync.dma_start(out=out_d[:], in_=outs[:])

    nc.compile()
    return nc


class SpmdRunner:
    def __init__(self, nc, n_cores):
        install_neuronx_cc_hook()
        self.nc = nc
        self.n_cores = n_cores
        partition_name = (nc.partition_id_tensor.name
                          if nc.partition_id_tensor else None)
        in_names, out_names, out_avals, zero_outs = [], [], [], []
        for alloc in nc.m.functions[0].allocations:
            if not isinstance(alloc, mybir.MemoryLocationSet):
                continue
            name = alloc.memorylocations[0].name
            if alloc.kind == "ExternalInput":
                if name != partition_name:
                    in_names.append(name)
            elif alloc.kind == "ExternalOutput":
                shape = tuple(alloc.tensor_shape)
                dt = mybir.dt.np(alloc.dtype)
                out_names.append(name)
                out_avals.append(jax.core.ShapedArray(shape, dt))
                zero_outs.append(np.zeros(shape, dt))
        self.in_names, self.out_names = in_names, out_names
        self.zero_outs = zero_outs
        bind_in_names = in_names + out_names
        if partition_name is not None:
            bind_in_names.append(partition_name)

        def _body(*args):
            operands = list(args)
            if partition_name is not None:
                operands.append(bass2jax.partition_id_tensor())
            outs = _bass_exec_p.bind(
                *operands,
                out_avals=tuple(out_avals),
                in_names=tuple(bind_in_names),
                out_names=tuple(out_names),
                lowering_input_output_aliases=(),
                sim_require_finite=False,
                sim_require_nnan=False,
                nc=nc,
            )
            return tuple(outs)

        devices = jax.devices()[:n_cores]
        self.mesh = Mesh(np.asarray(devices), ("core",))
        n_args = len(in_names) + len(zero_outs)
        in_specs = (PartitionSpec("core"),) * n_args
        out_specs = (PartitionSpec("core"),) * len(out_names)
        self.fn = jax.jit(
            shard_map(_body, mesh=self.mesh, in_specs=in_specs,
                      out_specs=out_specs, check_rep=False),
            keep_unused=True,
        )
        self._dev_in = None

    def set_inputs(self, in_maps):
        assert len(in_maps) == self.n_cores
        concat = [np.concatenate([np.asarray(in_maps[c][n])
                                  for c in range(self.n_cores)], axis=0)
                  for n in self.in_names]
        self._dev_in = [jax.device_put(a) for a in concat]
        self._dev_zeros = [
            jax.device_put(np.zeros((self.n_cores * z.shape[0], *z.shape[1:]),
                                    z.dtype)) for z in self.zero_outs]
        jax.block_until_ready(self._dev_in)

    def run(self):
        outs = self.fn(*self._dev_in, *self._dev_zeros)
        jax.block_until_ready(outs)
        return outs

    def results(self, outs):
        res = [dict() for _ in range(self.n_cores)]
        for i, name in enumerate(self.out_names):
            arr = np.asarray(outs[i])
            per = np.split(arr, self.n_cores, axis=0)
            for c in range(self.n_cores):
                res[c][name] = per[c]
        return res


_CACHE = {}


def _get_runner(meta, in_maps, repeats=1):
    key = (tuple(sorted((k, v) for k, v in meta.__dict__.items()
                        if not isinstance(v, tuple))),
           meta.tbb, meta.toff, repeats)
    if key not in _CACHE:
        nc = build_nc(meta, repeats=repeats)
        _CACHE[key] = SpmdRunner(nc, meta.C)
    return _CACHE[key]


def kernel(x, edge_index, batch, W_emb, b_emb, conv_W, conv_b,
           W1, b1, W2, b2, W3, b3):
    """Full (unsharded) inputs -> full [G, 1] float32 output."""
    G = 256
    meta, in_maps = preprocess(
        x, edge_index, batch, W_emb, b_emb, conv_W, conv_b,
        W1, b1, W2, b2, W3, b3, n_cores=8, G=G)
    r = _get_runner(meta, in_maps)
    r.set_inputs(in_maps)
    res = r.results(r.run())
    return np.ascontiguousarray(res[0]["out"].reshape(G, 1).astype(np.float32))
